# revision 2
# baseline (speedup 1.0000x reference)
"""BiLSTM (T=2048, B=32, I=H=256) Bass kernel for 8 NeuronCores.

Sharding (data-parallel per the hint): batch is split 8 ways; every core
runs BOTH directions for its 4 batch lanes as two independent chains so
each chain's cross-engine latency is hidden by the other chain's work.
The backward direction consumes x flipped along time AND batch (faithful
to torch.flip(input_, [0,1])).

On-chip layout is fully transposed ([H partitions, lanes free]) so the
sequential scan needs no per-step transposes. The recurrent step is
PE-issue-bound on HW (~44ns/matmul instr x 34 instrs/step), so the
elementwise path is cut to the minimum: a sigmoid-only formulation

    store  h~ = h/2   (Whh pre-scaled by 2 on host)
    store  C  = 2c    (c0 pre-scaled, output halved on host)
    gate rows permuted to [f,g,i,o], g rows pre-scaled by 2

    sig    = Sigmoid(gates)                      1 Act instr  [128,32]
    cf     = sig_f * C                           1 DVE tt
    u2     = (sig_g - 0.5) * sig_i  (= u/2)      1 DVE stt
    C'     = 4*u2 + cf              (= 2c')      1 DVE stt
    sigc   = Sigmoid(C')                         1 Act instr  [128,8]
    h~'    = (sigc - 0.5) * sig_o   (= h'/2)     1 DVE stt

which is algebraically exact (tanh(x) = 2*sigmoid(2x) - 1) and needs no
Tanh activation table. Weights/x/h run in fp16; c stays fp32 on chip.

Length masking is exact and handled on the host: a lane's post-length
steps compute garbage that never contaminates other lanes (lanes are
independent columns end to end; sigmoids keep values bounded), and the
output tail t >= len is overwritten host-side with the frozen value at
len-1 — identical to the reference's masked freeze.
"""

import sys

import numpy as np

# ---- problem constants (hardcoded per contract) ----
T, B, I, H = 2048, 32, 256, 256
NCORES = 8
ND = 2            # directions per core
BL = B // NCORES  # 4 batch lanes per core per direction
B2 = 2 * BL       # (H-tile, lane) free width of h/c state
G = 8             # 4H/128 gate row tiles, order [f0,f1,g0,g1,i0,i1,o0,o1]
KT = 2            # H/128 contraction tiles
TC = 128          # scan chunk length (steps per For_i iteration)
RPC = KT * 128    # chunk row stride shared by xarr0/1, h_out, c_out

_CACHE = {}


def _import_bass():
    try:
        import concourse.bass  # noqa: F401
    except ImportError:
        sys.path.insert(0, "/opt/trn_rl_repo")


def build_program(t_total=T, tc=TC,
                  skip_mm=False, skip_eltwise=False, sever_h=False):
    """Build the SPMD Bass program (identical on all cores)."""
    _import_bass()
    import concourse.bass as bass
    import concourse.mybir as mybir
    from concourse import bacc
    from concourse.tile import TileContext

    ds = bass.ds
    f32 = mybir.dt.float32
    f16 = mybir.dt.float16
    AF = mybir.ActivationFunctionType
    OP = mybir.AluOpType

    n_chunks = t_total // tc
    assert t_total % tc == 0

    nc = bacc.Bacc("TRN2", target_bir_lowering=False, debug=False,
                   num_devices=NCORES)

    # DRAM I/O. All chunked tensors share a 256-row-per-chunk stride so
    # one For_i induction variable addresses everything.
    xarr0 = nc.dram_tensor("xarr0", [n_chunks * RPC, tc * BL], f16,
                           kind="ExternalInput")
    xarr1 = nc.dram_tensor("xarr1", [n_chunks * RPC, tc * BL], f16,
                           kind="ExternalInput")
    whhT = nc.dram_tensor("whhT", [ND * KT * G * 128, 128], f16,
                          kind="ExternalInput")
    wihT = nc.dram_tensor("wihT", [ND * KT * G * 128, 128], f16,
                          kind="ExternalInput")
    biasT = nc.dram_tensor("biasT", [128, ND * G], f32, kind="ExternalInput")
    hc0T = nc.dram_tensor("hc0T", [128, ND * 2 * B2], f32,
                          kind="ExternalInput")
    identT = nc.dram_tensor("identT", [128, 128], f16, kind="ExternalInput")
    h_out = nc.dram_tensor("h_out", [n_chunks * RPC, tc * B2], f16,
                           kind="ExternalOutput")
    c_out = nc.dram_tensor("c_out", [n_chunks * RPC, tc * B2], f32,
                           kind="ExternalOutput")

    from contextlib import ExitStack
    with TileContext(nc) as tcx, ExitStack() as stk:
        wpool = stk.enter_context(tcx.tile_pool(name="weights", bufs=1))
        spool = stk.enter_context(tcx.tile_pool(name="state", bufs=1))
        xpool = stk.enter_context(tcx.tile_pool(name="xdata", bufs=1))
        tpool = stk.enter_context(tcx.tile_pool(name="temps", bufs=3))
        papool = stk.enter_context(tcx.tile_pool(name="psa", bufs=2,
                                                 space="PSUM"))
        pgpool = stk.enter_context(tcx.tile_pool(name="psg", bufs=2,
                                                 space="PSUM"))

        whh_sb = wpool.tile([128, ND * KT * G * 128], f16)
        wih_sb = wpool.tile([128, ND * KT * G * 128], f16)
        bias_sb = wpool.tile([128, ND * G], f32)
        hc0_sb = wpool.tile([128, ND * 2 * B2], f32)
        ident_sb = wpool.tile([128, 128], f16)
        HB = (tc + 1) * B2  # per-direction history block
        h_hist = spool.tile([128, ND * HB], f16)
        c_hist = spool.tile([128, ND * HB], f32)
        xp = [xpool.tile([128, G * tc * BL], f16, name=f"xp{d}")
              for d in range(ND)]
        xin = xpool.tile([128, ND * KT * tc * BL], f16)

        def w_sl(sb, d, ki, j):
            off = ((d * KT + ki) * G + j) * 128
            return sb[:, off:off + 128]

        def h_sl(d, slot, ki=0, w=None):
            off = d * HB + slot * B2 + ki * BL
            return h_hist[:, off:off + (w if w is not None else B2)]

        def c_sl(d, slot):
            off = d * HB + slot * B2
            return c_hist[:, off:off + B2]

        # --- load constants ---
        nc.sync.dma_start(
            out=whh_sb[:].rearrange("p (a m) -> p a m", m=128),
            in_=whhT.ap().rearrange("(a p) m -> p a m", p=128))
        nc.sync.dma_start(
            out=wih_sb[:].rearrange("p (a m) -> p a m", m=128),
            in_=wihT.ap().rearrange("(a p) m -> p a m", p=128))
        nc.sync.dma_start(out=bias_sb[:], in_=biasT.ap())
        nc.sync.dma_start(out=hc0_sb[:], in_=hc0T.ap())
        nc.sync.dma_start(out=ident_sb[:], in_=identT.ap())
        for d in range(ND):
            nc.vector.tensor_copy(h_sl(d, 0),
                                  hc0_sb[:, (2 * d) * B2:(2 * d + 1) * B2])
            nc.vector.tensor_copy(c_sl(d, 0),
                                  hc0_sb[:, (2 * d + 1) * B2:(2 * d + 2) * B2])

        def chunk_body(kbase):
            # 1) DMA x.T chunk in, one transfer per direction
            for d, xa in ((0, xarr0), (1, xarr1)):
                nc.sync.dma_start(
                    out=xin[:, d * KT * tc * BL:(d + 1) * KT * tc * BL]
                        .rearrange("p (a n) -> p a n", a=KT),
                    in_=xa.ap()[ds(kbase, RPC), :]
                        .rearrange("(a p) n -> p a n", p=128))
            # 2) Phase A: xp[d] = Wih_perm @ x.T + bias, laid out (j, t, l)
            PA_N = tc * BL
            for d in range(ND):
                for j in range(G):
                    ps = papool.tile([128, PA_N], f32, tag=f"pa{d}",
                                     name="psa")
                    for ki in range(KT):
                        a = d * KT + ki
                        nc.tensor.matmul(
                            ps[:], w_sl(wih_sb, d, ki, j),
                            xin[:, a * PA_N:(a + 1) * PA_N],
                            start=(ki == 0), stop=(ki == KT - 1))
                    bcol = bias_sb[:, d * G + j:d * G + j + 1]
                    dst = xp[d][:, j * PA_N:(j + 1) * PA_N]
                    if j % 2 == 0:
                        nc.scalar.activation(dst, ps[:], AF.Identity,
                                             bias=bcol)
                    else:
                        nc.vector.tensor_scalar(dst, ps[:], bcol, None,
                                                OP.add)
            # 3) sequential scan, two chains (directions) interleaved
            for tl in range(tc):
                tj = 0 if sever_h else tl
                psg = [None, None]
                xpv = [xp[d][:].rearrange("p (g t l) -> p g t l",
                                          g=G, l=BL)[:, :, tl, :]
                       for d in range(ND)]
                for d in range(ND):
                    if skip_mm:
                        continue
                    ps = pgpool.tile([128, G * BL], f32, tag=f"g{d}",
                                     name="psg")
                    psg[d] = ps
                    # inject xp into the gate bank ahead of the h MMs
                    nc.tensor.matmul(
                        ps[:].rearrange("p (g l) -> p g l", l=BL),
                        ident_sb[:], xpv[d],
                        start=True, stop=False, skip_group_check=True)
                    for j in range(G):
                        for ki in range(KT):
                            nc.tensor.matmul(
                                ps[:, j * BL:(j + 1) * BL],
                                w_sl(whh_sb, d, ki, j),
                                h_sl(d, tj, ki, BL),
                                start=False,
                                stop=(ki == KT - 1 and j == G - 1),
                                skip_group_check=True)
                if skip_eltwise:
                    continue
                sig, sigc, cf, u2 = [], [], [], []
                for d in range(ND):
                    sig.append(tpool.tile([128, G * BL], f32, tag=f"sg{d}",
                                          name="sig"))
                    sigc.append(tpool.tile([128, B2], f32, tag=f"sc{d}",
                                           name="sigc"))
                    cf.append(tpool.tile([128, B2], f32, tag=f"cf{d}",
                                         name="cf"))
                    u2.append(tpool.tile([128, B2], f32, tag=f"u{d}",
                                         name="u2"))
                src = [xpv[d] if skip_mm else psg[d][:] for d in range(ND)]
                # emission order keeps each engine's in-order queue free of
                # head-of-line blocking across the two chains:
                #   Act: sg0, sg1, sc0, sc1   DVE: cf0,u20,C'0, cf1,u21,C'1,
                #   h20, h21
                nc.scalar.activation(sig[0][:], src[0], AF.Sigmoid)
                nc.vector.tensor_mul(cf[0][:], sig[0][:, 0:B2], c_sl(0, tl))
                nc.vector.scalar_tensor_tensor(
                    u2[0][:], sig[0][:, B2:2 * B2], 0.5,
                    sig[0][:, 2 * B2:3 * B2], OP.subtract, OP.mult)
                nc.vector.scalar_tensor_tensor(
                    c_sl(0, tl + 1), u2[0][:], 4.0, cf[0][:],
                    OP.mult, OP.add)
                nc.scalar.activation(sig[1][:], src[1], AF.Sigmoid)
                nc.scalar.activation(sigc[0][:], c_sl(0, tl + 1), AF.Sigmoid)
                nc.vector.tensor_mul(cf[1][:], sig[1][:, 0:B2], c_sl(1, tl))
                nc.vector.scalar_tensor_tensor(
                    u2[1][:], sig[1][:, B2:2 * B2], 0.5,
                    sig[1][:, 2 * B2:3 * B2], OP.subtract, OP.mult)
                nc.vector.scalar_tensor_tensor(
                    c_sl(1, tl + 1), u2[1][:], 4.0, cf[1][:],
                    OP.mult, OP.add)
                nc.vector.scalar_tensor_tensor(
                    h_sl(0, tl + 1), sigc[0][:], 0.5,
                    sig[0][:, 3 * B2:4 * B2], OP.subtract, OP.mult)
                nc.scalar.activation(sigc[1][:], c_sl(1, tl + 1), AF.Sigmoid)
                nc.vector.scalar_tensor_tensor(
                    h_sl(1, tl + 1), sigc[1][:], 0.5,
                    sig[1][:, 3 * B2:4 * B2], OP.subtract, OP.mult)
            # 4) flush chunk outputs (one DMA each for h and c), carry state
            nc.sync.dma_start(
                out=h_out.ap()[ds(kbase, RPC), :]
                    .rearrange("(a p) n -> p a n", p=128),
                in_=h_hist[:].rearrange("p (a n) -> p a n", a=ND)[:, :, B2:])
            nc.sync.dma_start(
                out=c_out.ap()[ds(kbase, RPC), :]
                    .rearrange("(a p) n -> p a n", p=128),
                in_=c_hist[:].rearrange("p (a n) -> p a n", a=ND)[:, :, B2:])
            for d in range(ND):
                nc.vector.tensor_copy(h_sl(d, 0), h_sl(d, tc))
                nc.vector.tensor_copy(c_sl(d, 0), c_sl(d, tc))

        if n_chunks == 1:
            chunk_body(0)
        else:
            import concourse.mybir as _mb
            with tcx.For_i(0, n_chunks * RPC, RPC,
                           hint_engines=(_mb.EngineType.PE,
                                         _mb.EngineType.Activation,
                                         _mb.EngineType.DVE)) as kbase:
                chunk_body(kbase)

    nc.compile()
    return nc


# ---------------- host-side data marshalling ----------------

def _perm_scale_rows(w):
    """Reorder gate rows [i,f,g,o] -> [f,g,i,o], scale g rows by 2."""
    return np.concatenate(
        [w[256:512], 2.0 * w[512:768], w[0:256], w[768:1024]], 0)


def prep_inputs(x, length, h0, c0, Wih_f, Whh_f, bih_f, bhh_f,
                Wih_b, Whh_b, bih_b, bhh_b, t_total=T, tc=TC):
    """Build per-core input dicts."""
    n_chunks = t_total // tc
    dt = np.float16
    x = np.asarray(x, np.float32)
    x_b = x[::-1, ::-1, :]

    # Whh additionally scaled by 2: the h state is stored as h/2.
    wihP = {0: _perm_scale_rows(np.asarray(Wih_f)),
            1: _perm_scale_rows(np.asarray(Wih_b))}
    whhP = {0: 2.0 * _perm_scale_rows(np.asarray(Whh_f)),
            1: 2.0 * _perm_scale_rows(np.asarray(Whh_b))}
    biasP = {0: _perm_scale_rows(
                 (np.asarray(bih_f) + np.asarray(bhh_f))[:, None]),
             1: _perm_scale_rows(
                 (np.asarray(bih_b) + np.asarray(bhh_b))[:, None])}

    def wtiles(w):
        out = np.empty((ND * KT * G * 128, 128), dt)
        for d in range(ND):
            wT = w[d].T.astype(dt)
            for ki in range(KT):
                for j in range(G):
                    off = ((d * KT + ki) * G + j) * 128
                    out[off:off + 128] = wT[ki * 128:(ki + 1) * 128,
                                            j * 128:(j + 1) * 128]
        return out

    whhT = wtiles(whhP)
    wihT = wtiles(wihP)
    biasT = np.zeros((128, ND * G), np.float32)
    for d in range(ND):
        for j in range(G):
            biasT[:, d * G + j] = biasP[d][j * 128:(j + 1) * 128, 0]

    h0 = np.asarray(h0, np.float32)
    c0 = np.asarray(c0, np.float32)

    in_maps = []
    for core in range(NCORES):
        sl = slice(core * BL, (core + 1) * BL)
        xarr = {}
        for d, xd in ((0, x), (1, x_b)):
            xs = xd[:t_total, sl, :]
            xT = np.ascontiguousarray(xs.transpose(0, 2, 1)).astype(dt)
            xa = np.empty((n_chunks * RPC, tc * BL), dt)
            for k in range(n_chunks):
                for ki in range(KT):
                    roff = k * RPC + ki * 128
                    blk = xT[k * tc:(k + 1) * tc,
                             ki * 128:(ki + 1) * 128, :]
                    xa[roff:roff + 128] = (
                        blk.transpose(1, 0, 2).reshape(128, tc * BL))
            xarr[d] = xa
        hc0T = np.zeros((128, ND * 2 * B2), np.float32)
        for d in range(ND):
            for s, st in ((0, 0.5 * h0), (1, 2.0 * c0)):
                stT = st[sl].T
                for ki in range(KT):
                    off = (2 * d + s) * B2 + ki * BL
                    hc0T[:, off:off + BL] = stT[ki * 128:(ki + 1) * 128, :]
        in_maps.append({"xarr0": xarr[0], "xarr1": xarr[1], "whhT": whhT,
                        "wihT": wihT, "biasT": biasT, "hc0T": hc0T,
                        "identT": np.eye(128, dtype=dt)})
    return in_maps


def assemble_outputs(results, length, t_total=T, tc=TC):
    """results: per-core {'h_out','c_out'}. Returns (output, cell)."""
    n_chunks = t_total // tc
    length = np.asarray(length)
    out_h = np.empty((t_total, 2 * B, H), np.float32)
    out_c = np.empty((t_total, 2 * B, H), np.float32)
    for core in range(NCORES):
        sl = slice(core * BL, (core + 1) * BL)
        for arr, out, scale in ((results[core]["h_out"], out_h, 2.0),
                                (results[core]["c_out"], out_c, 0.5)):
            v = arr.astype(np.float32).reshape(n_chunks, ND, 128, tc, KT, BL)
            # [k, d, p, tl, ki, l] -> [d, (k tl), l, (ki p)]
            v = v.transpose(1, 0, 3, 5, 4, 2).reshape(ND, t_total, BL, H)
            for d in range(ND):
                col0 = d * B + sl.start
                out[:, col0:col0 + BL, :] = scale * v[d]
    for b in range(B):
        ln = int(length[b])
        if ln < t_total:
            out_h[ln:, b] = out_h[ln - 1, b]
            out_c[ln:, b] = out_c[ln - 1, b]
            out_h[ln:, B + b] = out_h[ln - 1, B + b]
            out_c[ln:, B + b] = out_c[ln - 1, B + b]
    return out_h, out_c


def kernel(**inputs):
    _import_bass()
    from concourse.bass_utils import run_bass_kernel_spmd
    key = (T, TC)
    if key not in _CACHE:
        _CACHE[key] = build_program(T, TC)
    nc = _CACHE[key]
    in_maps = prep_inputs(**inputs)
    res = run_bass_kernel_spmd(nc, in_maps, list(range(NCORES)))
    return assemble_outputs(res.results, inputs["length"])


# revision 9
# speedup vs baseline: 1.4204x; 1.4204x over previous
"""BiLSTM (T=2048, B=32, I=H=256) Bass kernel for 8 NeuronCores.

Sharding (data-parallel per the hint): batch is split 8 ways; every core
runs BOTH directions for its 4 batch lanes as two independent chains so
each chain's cross-engine latency is hidden by the other chain's work.
The backward direction consumes x flipped along time AND batch (faithful
to torch.flip(input_, [0,1])).

On-chip layout is fully transposed ([H partitions, lanes free]) so the
sequential scan needs no per-step transposes. The recurrent step is
PE-issue-bound on HW (~44ns/matmul instr x 34 instrs/step), so the
elementwise path is cut to the minimum: a sigmoid-only formulation

    store  h~ = h/2   (Whh pre-scaled by 2 on host)
    store  C  = 2c    (c0 pre-scaled, output halved on host)
    gate rows permuted to [f,g,i,o], g rows pre-scaled by 2

    sig    = Sigmoid(gates)                      1 Act instr  [128,32]
    cf     = sig_f * C                           1 DVE tt
    u2     = (sig_g - 0.5) * sig_i  (= u/2)      1 DVE stt
    C'     = 4*u2 + cf              (= 2c')      1 DVE stt
    sigc   = Sigmoid(C')                         1 Act instr  [128,8]
    h~'    = (sigc - 0.5) * sig_o   (= h'/2)     1 DVE stt

which is algebraically exact (tanh(x) = 2*sigmoid(2x) - 1) and needs no
Tanh activation table. Weights/x/h run in fp16; c stays fp32 on chip.

Length masking is exact and handled on the host: a lane's post-length
steps compute garbage that never contaminates other lanes (lanes are
independent columns end to end; sigmoids keep values bounded), and the
output tail t >= len is overwritten host-side with the frozen value at
len-1 — identical to the reference's masked freeze.
"""

import sys

import numpy as np

# ---- problem constants (hardcoded per contract) ----
T, B, I, H = 2048, 32, 256, 256
NCORES = 8
ND = 2            # directions per core
BL = B // NCORES  # 4 batch lanes per core per direction
B2 = 2 * BL       # (H-tile, lane) free width of h/c state
G = 8             # 4H/128 gate row tiles, order [f0,f1,g0,g1,i0,i1,o0,o1]
KT = 2            # H/128 contraction tiles
TC = 128          # scan chunk length (steps per For_i iteration)
RPC = KT * 128    # chunk row stride shared by xarr0/1, h_out, c_out

_CACHE = {}


def _import_bass():
    try:
        import concourse.bass  # noqa: F401
    except ImportError:
        sys.path.insert(0, "/opt/trn_rl_repo")


def build_program(t_total=T, tc=TC,
                  skip_mm=False, skip_eltwise=False, sever_h=False,
                  dve_order=1, split_h2=False, u2_first=True, o_last=False):
    """Build the SPMD Bass program (identical on all cores)."""
    _import_bass()
    import concourse.bass as bass
    import concourse.mybir as mybir
    from concourse import bacc
    from concourse.tile import TileContext

    ds = bass.ds
    f32 = mybir.dt.float32
    f16 = mybir.dt.float16
    AF = mybir.ActivationFunctionType
    OP = mybir.AluOpType

    n_chunks = t_total // tc
    assert t_total % tc == 0

    nc = bacc.Bacc("TRN2", target_bir_lowering=False, debug=False,
                   num_devices=NCORES)

    # DRAM I/O. All chunked tensors share a 256-row-per-chunk stride so
    # one For_i induction variable addresses everything.
    xarr0 = nc.dram_tensor("xarr0", [n_chunks * RPC, tc * BL], f16,
                           kind="ExternalInput")
    xarr1 = nc.dram_tensor("xarr1", [n_chunks * RPC, tc * BL], f16,
                           kind="ExternalInput")
    whhT = nc.dram_tensor("whhT", [ND * KT * G * 128, 128], f16,
                          kind="ExternalInput")
    wihT = nc.dram_tensor("wihT", [ND * KT * G * 128, 128], f16,
                          kind="ExternalInput")
    biasT = nc.dram_tensor("biasT", [128, ND * G], f32, kind="ExternalInput")
    hc0T = nc.dram_tensor("hc0T", [128, ND * 2 * B2], f32,
                          kind="ExternalInput")
    identT = nc.dram_tensor("identT", [128, 128], f16, kind="ExternalInput")
    h_out = nc.dram_tensor("h_out", [n_chunks * RPC, tc * B2], f16,
                           kind="ExternalOutput")
    c_out = nc.dram_tensor("c_out", [n_chunks * RPC, tc * B2], f32,
                           kind="ExternalOutput")

    from contextlib import ExitStack
    with TileContext(nc) as tcx, ExitStack() as stk:
        wpool = stk.enter_context(tcx.tile_pool(name="weights", bufs=1))
        spool = stk.enter_context(tcx.tile_pool(name="state", bufs=1))
        xpool = stk.enter_context(tcx.tile_pool(name="xdata", bufs=1))
        tpool = stk.enter_context(tcx.tile_pool(name="temps", bufs=3))
        papool = stk.enter_context(tcx.tile_pool(name="psa", bufs=2,
                                                 space="PSUM"))
        pgpool = stk.enter_context(tcx.tile_pool(name="psg", bufs=2,
                                                 space="PSUM"))

        whh_sb = wpool.tile([128, ND * KT * G * 128], f16)
        wih_sb = wpool.tile([128, ND * KT * G * 128], f16)
        bias_sb = wpool.tile([128, ND * G], f32)
        hc0_sb = wpool.tile([128, ND * 2 * B2], f32)
        ident_sb = wpool.tile([128, 128], f16)
        HB = (tc + 1) * B2  # per-direction history block
        h_hist = spool.tile([128, ND * HB], f16)
        c_hist = spool.tile([128, ND * HB], f32)
        xp = [xpool.tile([128, G * tc * BL], f16, name=f"xp{d}")
              for d in range(ND)]
        xin = xpool.tile([128, ND * KT * tc * BL], f16)

        def w_sl(sb, d, ki, j):
            off = ((d * KT + ki) * G + j) * 128
            return sb[:, off:off + 128]

        def h_sl(d, slot, ki=0, w=None):
            off = d * HB + slot * B2 + ki * BL
            return h_hist[:, off:off + (w if w is not None else B2)]

        def c_sl(d, slot):
            off = d * HB + slot * B2
            return c_hist[:, off:off + B2]

        # --- load constants ---
        nc.sync.dma_start(
            out=whh_sb[:].rearrange("p (a m) -> p a m", m=128),
            in_=whhT.ap().rearrange("(a p) m -> p a m", p=128))
        nc.sync.dma_start(
            out=wih_sb[:].rearrange("p (a m) -> p a m", m=128),
            in_=wihT.ap().rearrange("(a p) m -> p a m", p=128))
        nc.sync.dma_start(out=bias_sb[:], in_=biasT.ap())
        nc.sync.dma_start(out=hc0_sb[:], in_=hc0T.ap())
        nc.sync.dma_start(out=ident_sb[:], in_=identT.ap())
        for d in range(ND):
            nc.vector.tensor_copy(h_sl(d, 0),
                                  hc0_sb[:, (2 * d) * B2:(2 * d + 1) * B2])
            nc.vector.tensor_copy(c_sl(d, 0),
                                  hc0_sb[:, (2 * d + 1) * B2:(2 * d + 2) * B2])

        def chunk_body(kbase):
            # 1) DMA x.T chunk in, one transfer per direction
            for d, xa in ((0, xarr0), (1, xarr1)):
                nc.sync.dma_start(
                    out=xin[:, d * KT * tc * BL:(d + 1) * KT * tc * BL]
                        .rearrange("p (a n) -> p a n", a=KT),
                    in_=xa.ap()[ds(kbase, RPC), :]
                        .rearrange("(a p) n -> p a n", p=128))
            # 2) Phase A: xp[d] = Wih_perm @ x.T + bias, laid out (j, t, l)
            PA_N = tc * BL
            for d in range(ND):
                for j in range(G):
                    ps = papool.tile([128, PA_N], f32, tag=f"pa{d}",
                                     name="psa")
                    for ki in range(KT):
                        a = d * KT + ki
                        nc.tensor.matmul(
                            ps[:], w_sl(wih_sb, d, ki, j),
                            xin[:, a * PA_N:(a + 1) * PA_N],
                            start=(ki == 0), stop=(ki == KT - 1))
                    bcol = bias_sb[:, d * G + j:d * G + j + 1]
                    dst = xp[d][:, j * PA_N:(j + 1) * PA_N]
                    if j % 2 == 0:
                        nc.scalar.activation(dst, ps[:], AF.Identity,
                                             bias=bcol)
                    else:
                        nc.vector.tensor_scalar(dst, ps[:], bcol, None,
                                                OP.add)
            # 3) sequential scan, two chains (directions) interleaved
            for tl in range(tc):
                tj = 0 if sever_h else tl
                psg = [None, None]
                xpv = [xp[d][:].rearrange("p (g t l) -> p g t l",
                                          g=G, l=BL)[:, :, tl, :]
                       for d in range(ND)]
                for d in range(ND):
                    if skip_mm:
                        continue
                    ps = pgpool.tile([128, G * BL], f32, tag=f"g{d}",
                                     name="psg")
                    psg[d] = ps
                    # inject xp into the gate bank ahead of the h MMs
                    nc.tensor.matmul(
                        ps[:].rearrange("p (g l) -> p g l", l=BL),
                        ident_sb[:], xpv[d],
                        start=True, stop=False, skip_group_check=True)
                    for j in range(G):
                        for ki in range(KT):
                            nc.tensor.matmul(
                                ps[:, j * BL:(j + 1) * BL],
                                w_sl(whh_sb, d, ki, j),
                                h_sl(d, tj, ki, BL),
                                start=False,
                                stop=(ki == KT - 1 and j == G - 1),
                                skip_group_check=True)
                if skip_eltwise:
                    continue
                sig, sigc, cf, u2 = [], [], [], []
                for d in range(ND):
                    sig.append(tpool.tile([128, G * BL], f32, tag=f"sg{d}",
                                          name="sig"))
                    sigc.append(tpool.tile([128, B2], f32, tag=f"sc{d}",
                                           name="sigc"))
                    cf.append(tpool.tile([128, B2], f32, tag=f"cf{d}",
                                         name="cf"))
                    u2.append(tpool.tile([128, B2], f32, tag=f"u{d}",
                                         name="u2"))
                src = [xpv[d] if skip_mm else psg[d][:] for d in range(ND)]

                def emit_sg(d):
                    if o_last and not skip_mm:
                        nc.scalar.activation(sig[d][:, 0:3 * B2],
                                             src[d][:, 0:3 * B2], AF.Sigmoid)
                        nc.scalar.activation(sig[d][:, 3 * B2:4 * B2],
                                             src[d][:, 3 * B2:4 * B2],
                                             AF.Sigmoid)
                    else:
                        nc.scalar.activation(sig[d][:], src[d], AF.Sigmoid)

                def emit_cchain(d):
                    def _cf():
                        nc.vector.tensor_mul(cf[d][:], sig[d][:, 0:B2],
                                             c_sl(d, tl))

                    def _u2():
                        nc.vector.scalar_tensor_tensor(
                            u2[d][:], sig[d][:, B2:2 * B2], 0.5,
                            sig[d][:, 2 * B2:3 * B2], OP.subtract, OP.mult)

                    if u2_first:
                        _u2(); _cf()
                    else:
                        _cf(); _u2()
                    nc.vector.scalar_tensor_tensor(
                        c_sl(d, tl + 1), u2[d][:], 4.0, cf[d][:],
                        OP.mult, OP.add)

                def emit_sc(d):
                    nc.scalar.activation(sigc[d][:], c_sl(d, tl + 1),
                                         AF.Sigmoid)

                def emit_h2(d):
                    if split_h2:
                        for ki in range(KT):
                            nc.vector.scalar_tensor_tensor(
                                h_sl(d, tl + 1, ki, BL),
                                sigc[d][:, ki * BL:(ki + 1) * BL], 0.5,
                                sig[d][:, 3 * B2 + ki * BL:
                                     3 * B2 + (ki + 1) * BL],
                                OP.subtract, OP.mult)
                    else:
                        nc.vector.scalar_tensor_tensor(
                            h_sl(d, tl + 1), sigc[d][:], 0.5,
                            sig[d][:, 3 * B2:4 * B2], OP.subtract, OP.mult)

                if dve_order == 0:
                    # DVE: cf0,u20,C'0, cf1,u21,C'1, h20, h21
                    emit_sg(0); emit_cchain(0); emit_sg(1); emit_sc(0)
                    emit_cchain(1); emit_h2(0); emit_sc(1); emit_h2(1)
                else:
                    # DVE: cf0,u20,C'0, h20, cf1,u21,C'1, h21
                    emit_sg(0); emit_cchain(0); emit_sg(1); emit_sc(0)
                    emit_h2(0); emit_cchain(1); emit_sc(1); emit_h2(1)
            # 4) flush chunk outputs (one DMA each for h and c), carry state
            nc.sync.dma_start(
                out=h_out.ap()[ds(kbase, RPC), :]
                    .rearrange("(a p) n -> p a n", p=128),
                in_=h_hist[:].rearrange("p (a n) -> p a n", a=ND)[:, :, B2:])
            nc.sync.dma_start(
                out=c_out.ap()[ds(kbase, RPC), :]
                    .rearrange("(a p) n -> p a n", p=128),
                in_=c_hist[:].rearrange("p (a n) -> p a n", a=ND)[:, :, B2:])
            for d in range(ND):
                nc.vector.tensor_copy(h_sl(d, 0), h_sl(d, tc))
                nc.vector.tensor_copy(c_sl(d, 0), c_sl(d, tc))

        if n_chunks == 1:
            chunk_body(0)
        else:
            import concourse.mybir as _mb
            with tcx.For_i(0, n_chunks * RPC, RPC,
                           hint_engines=(_mb.EngineType.PE,
                                         _mb.EngineType.Activation,
                                         _mb.EngineType.DVE)) as kbase:
                chunk_body(kbase)

    nc.compile()
    return nc


# ---------------- host-side data marshalling ----------------

def _perm_scale_rows(w):
    """Reorder gate rows [i,f,g,o] -> [f,g,i,o], scale g rows by 2."""
    return np.concatenate(
        [w[256:512], 2.0 * w[512:768], w[0:256], w[768:1024]], 0)


def prep_inputs(x, length, h0, c0, Wih_f, Whh_f, bih_f, bhh_f,
                Wih_b, Whh_b, bih_b, bhh_b, t_total=T, tc=TC):
    """Build per-core input dicts."""
    n_chunks = t_total // tc
    dt = np.float16
    x = np.asarray(x, np.float32)
    x_b = x[::-1, ::-1, :]

    # Whh additionally scaled by 2: the h state is stored as h/2.
    wihP = {0: _perm_scale_rows(np.asarray(Wih_f)),
            1: _perm_scale_rows(np.asarray(Wih_b))}
    whhP = {0: 2.0 * _perm_scale_rows(np.asarray(Whh_f)),
            1: 2.0 * _perm_scale_rows(np.asarray(Whh_b))}
    biasP = {0: _perm_scale_rows(
                 (np.asarray(bih_f) + np.asarray(bhh_f))[:, None]),
             1: _perm_scale_rows(
                 (np.asarray(bih_b) + np.asarray(bhh_b))[:, None])}

    def wtiles(w):
        out = np.empty((ND * KT * G * 128, 128), dt)
        for d in range(ND):
            wT = w[d].T.astype(dt)
            for ki in range(KT):
                for j in range(G):
                    off = ((d * KT + ki) * G + j) * 128
                    out[off:off + 128] = wT[ki * 128:(ki + 1) * 128,
                                            j * 128:(j + 1) * 128]
        return out

    whhT = wtiles(whhP)
    wihT = wtiles(wihP)
    biasT = np.zeros((128, ND * G), np.float32)
    for d in range(ND):
        for j in range(G):
            biasT[:, d * G + j] = biasP[d][j * 128:(j + 1) * 128, 0]

    h0 = np.asarray(h0, np.float32)
    c0 = np.asarray(c0, np.float32)

    in_maps = []
    for core in range(NCORES):
        sl = slice(core * BL, (core + 1) * BL)
        xarr = {}
        for d, xd in ((0, x), (1, x_b)):
            xs = xd[:t_total, sl, :]
            xT = np.ascontiguousarray(xs.transpose(0, 2, 1)).astype(dt)
            xa = np.empty((n_chunks * RPC, tc * BL), dt)
            for k in range(n_chunks):
                for ki in range(KT):
                    roff = k * RPC + ki * 128
                    blk = xT[k * tc:(k + 1) * tc,
                             ki * 128:(ki + 1) * 128, :]
                    xa[roff:roff + 128] = (
                        blk.transpose(1, 0, 2).reshape(128, tc * BL))
            xarr[d] = xa
        hc0T = np.zeros((128, ND * 2 * B2), np.float32)
        for d in range(ND):
            for s, st in ((0, 0.5 * h0), (1, 2.0 * c0)):
                stT = st[sl].T
                for ki in range(KT):
                    off = (2 * d + s) * B2 + ki * BL
                    hc0T[:, off:off + BL] = stT[ki * 128:(ki + 1) * 128, :]
        in_maps.append({"xarr0": xarr[0], "xarr1": xarr[1], "whhT": whhT,
                        "wihT": wihT, "biasT": biasT, "hc0T": hc0T,
                        "identT": np.eye(128, dtype=dt)})
    return in_maps


def assemble_outputs(results, length, t_total=T, tc=TC):
    """results: per-core {'h_out','c_out'}. Returns (output, cell)."""
    n_chunks = t_total // tc
    length = np.asarray(length)
    out_h = np.empty((t_total, 2 * B, H), np.float32)
    out_c = np.empty((t_total, 2 * B, H), np.float32)
    for core in range(NCORES):
        sl = slice(core * BL, (core + 1) * BL)
        for arr, out, scale in ((results[core]["h_out"], out_h, 2.0),
                                (results[core]["c_out"], out_c, 0.5)):
            v = arr.astype(np.float32).reshape(n_chunks, ND, 128, tc, KT, BL)
            # [k, d, p, tl, ki, l] -> [d, (k tl), l, (ki p)]
            v = v.transpose(1, 0, 3, 5, 4, 2).reshape(ND, t_total, BL, H)
            for d in range(ND):
                col0 = d * B + sl.start
                out[:, col0:col0 + BL, :] = scale * v[d]
    for b in range(B):
        ln = int(length[b])
        if ln < t_total:
            out_h[ln:, b] = out_h[ln - 1, b]
            out_c[ln:, b] = out_c[ln - 1, b]
            out_h[ln:, B + b] = out_h[ln - 1, B + b]
            out_c[ln:, B + b] = out_c[ln - 1, B + b]
    return out_h, out_c


def kernel(**inputs):
    _import_bass()
    from concourse.bass_utils import run_bass_kernel_spmd
    key = (T, TC)
    if key not in _CACHE:
        _CACHE[key] = build_program(T, TC)
    nc = _CACHE[key]
    in_maps = prep_inputs(**inputs)
    res = run_bass_kernel_spmd(nc, in_maps, list(range(NCORES)))
    return assemble_outputs(res.results, inputs["length"])


# revision 14
# speedup vs baseline: 2.1945x; 1.5450x over previous
"""BiLSTM (T=2048, B=32, I=H=256) Bass kernel for 8 NeuronCores.

Sharding (data-parallel per the hint): batch is split 8 ways; every core
runs BOTH directions for its 4 batch lanes as two independent chains so
each chain's cross-engine latency is hidden by the other chain's work.
The backward direction consumes x flipped along time AND batch (faithful
to torch.flip(input_, [0,1])).

On-chip layout is fully transposed ([H partitions, lanes free]) so the
sequential scan needs no per-step transposes. The recurrent step is
PE-issue-bound on HW (~44ns/matmul instr x 34 instrs/step), so the
elementwise path is cut to the minimum: a sigmoid-only formulation

    store  h~ = h/2   (Whh pre-scaled by 2 on host)
    store  C  = 2c    (c0 pre-scaled, output halved on host)
    gate rows permuted to [f,g,i,o], g rows pre-scaled by 2

    sig    = Sigmoid(gates)                      1 Act instr  [128,32]
    cf     = sig_f * C                           1 DVE tt
    u2     = (sig_g - 0.5) * sig_i  (= u/2)      1 DVE stt
    C'     = 4*u2 + cf              (= 2c')      1 DVE stt
    sigc   = Sigmoid(C')                         1 Act instr  [128,8]
    h~'    = (sigc - 0.5) * sig_o   (= h'/2)     1 DVE stt

which is algebraically exact (tanh(x) = 2*sigmoid(2x) - 1) and needs no
Tanh activation table. Weights/x/h run in fp16; c stays fp32 on chip.

Length masking is exact and handled on the host: a lane's post-length
steps compute garbage that never contaminates other lanes (lanes are
independent columns end to end; sigmoids keep values bounded), and the
output tail t >= len is overwritten host-side with the frozen value at
len-1 — identical to the reference's masked freeze.
"""

import sys

import numpy as np

# ---- problem constants (hardcoded per contract) ----
T, B, I, H = 2048, 32, 256, 256
NCORES = 8
ND = 2            # directions per core
BL = B // NCORES  # 4 batch lanes per core per direction
B2 = 2 * BL       # (H-tile, lane) free width of h/c state
G = 8             # 4H/128 gate row tiles, order [f0,f1,g0,g1,i0,i1,o0,o1]
KT = 2            # H/128 contraction tiles
TC = 256          # scan chunk length (steps per For_i iteration)
RPC = KT * 128    # chunk row stride shared by xarr0/1, h_out, c_out

_CACHE = {}


def _import_bass():
    try:
        import concourse.bass  # noqa: F401
    except ImportError:
        sys.path.insert(0, "/opt/trn_rl_repo")


def build_program(t_total=T, tc=TC,
                  skip_mm=False, skip_eltwise=False, sever_h=False,
                  dve_order=1, split_h2=False, u2_first=True, o_last=False):
    """Build the SPMD Bass program (identical on all cores)."""
    _import_bass()
    import concourse.bass as bass
    import concourse.mybir as mybir
    from concourse import bacc
    from concourse.tile import TileContext

    ds = bass.ds
    f32 = mybir.dt.float32
    f16 = mybir.dt.float16
    AF = mybir.ActivationFunctionType
    OP = mybir.AluOpType

    n_chunks = t_total // tc
    assert t_total % tc == 0

    nc = bacc.Bacc("TRN2", target_bir_lowering=False, debug=False,
                   num_devices=NCORES)

    # DRAM I/O. All chunked tensors share a 256-row-per-chunk stride so
    # one For_i induction variable addresses everything.
    xarr0 = nc.dram_tensor("xarr0", [n_chunks * RPC, tc * BL], f16,
                           kind="ExternalInput")
    xarr1 = nc.dram_tensor("xarr1", [n_chunks * RPC, tc * BL], f16,
                           kind="ExternalInput")
    whhT = nc.dram_tensor("whhT", [ND * KT * G * 128, 128], f16,
                          kind="ExternalInput")
    wihT = nc.dram_tensor("wihT", [ND * KT * G * 128, 128], f16,
                          kind="ExternalInput")
    biasT = nc.dram_tensor("biasT", [128, ND * G], f32, kind="ExternalInput")
    hc0T = nc.dram_tensor("hc0T", [128, ND * 2 * B2], f32,
                          kind="ExternalInput")
    identT = nc.dram_tensor("identT", [128, 128], f16, kind="ExternalInput")
    h_out = nc.dram_tensor("h_out", [n_chunks * RPC, tc * B2], f16,
                           kind="ExternalOutput")
    c_out = nc.dram_tensor("c_out", [n_chunks * RPC, tc * B2], f32,
                           kind="ExternalOutput")

    from contextlib import ExitStack
    with TileContext(nc) as tcx, ExitStack() as stk:
        wpool = stk.enter_context(tcx.tile_pool(name="weights", bufs=1))
        spool = stk.enter_context(tcx.tile_pool(name="state", bufs=1))
        xpool = stk.enter_context(tcx.tile_pool(name="xdata", bufs=1))
        tpool = stk.enter_context(tcx.tile_pool(name="temps", bufs=3))
        papool = stk.enter_context(tcx.tile_pool(name="psa", bufs=2,
                                                 space="PSUM"))
        pgpool = stk.enter_context(tcx.tile_pool(name="psg", bufs=2,
                                                 space="PSUM"))

        whh_sb = wpool.tile([128, ND * KT * G * 128], f16)
        wih_sb = wpool.tile([128, ND * KT * G * 128], f16)
        bias_sb = wpool.tile([128, ND * G], f32)
        hc0_sb = wpool.tile([128, ND * 2 * B2], f32)
        ident_sb = wpool.tile([128, 128], f16)
        HB = (tc + 1) * B2  # per-direction history block
        h_hist = spool.tile([128, ND * HB], f16)
        c_hist = spool.tile([128, ND * HB], f32)
        xp = [xpool.tile([128, G * tc * BL], f16, name=f"xp{d}")
              for d in range(ND)]
        xin = xpool.tile([128, ND * KT * tc * BL], f16)

        def w_sl(sb, d, ki, j):
            off = ((d * KT + ki) * G + j) * 128
            return sb[:, off:off + 128]

        def h_sl(d, slot, ki=0, w=None):
            off = d * HB + slot * B2 + ki * BL
            return h_hist[:, off:off + (w if w is not None else B2)]

        def c_sl(d, slot):
            off = d * HB + slot * B2
            return c_hist[:, off:off + B2]

        # --- load constants ---
        nc.sync.dma_start(
            out=whh_sb[:].rearrange("p (a m) -> p a m", m=128),
            in_=whhT.ap().rearrange("(a p) m -> p a m", p=128))
        nc.sync.dma_start(
            out=wih_sb[:].rearrange("p (a m) -> p a m", m=128),
            in_=wihT.ap().rearrange("(a p) m -> p a m", p=128))
        nc.sync.dma_start(out=bias_sb[:], in_=biasT.ap())
        nc.sync.dma_start(out=hc0_sb[:], in_=hc0T.ap())
        nc.sync.dma_start(out=ident_sb[:], in_=identT.ap())
        for d in range(ND):
            nc.vector.tensor_copy(h_sl(d, 0),
                                  hc0_sb[:, (2 * d) * B2:(2 * d + 1) * B2])
            nc.vector.tensor_copy(c_sl(d, 0),
                                  hc0_sb[:, (2 * d + 1) * B2:(2 * d + 2) * B2])

        def chunk_body(kbase):
            # 1) DMA x.T chunk in, one transfer per direction
            for d, xa in ((0, xarr0), (1, xarr1)):
                nc.sync.dma_start(
                    out=xin[:, d * KT * tc * BL:(d + 1) * KT * tc * BL]
                        .rearrange("p (a n) -> p a n", a=KT),
                    in_=xa.ap()[ds(kbase, RPC), :]
                        .rearrange("(a p) n -> p a n", p=128))
            # 2) Phase A: xp[d] = Wih_perm @ x.T + bias, laid out (j, t, l)
            PA_N = min(tc * BL, 512)
            for d in range(ND):
                for j in range(G):
                    for hf in range(tc * BL // PA_N):
                        ps = papool.tile([128, PA_N], f32, tag=f"pa{d}",
                                         name="psa")
                        for ki in range(KT):
                            a = d * KT + ki
                            nc.tensor.matmul(
                                ps[:], w_sl(wih_sb, d, ki, j),
                                xin[:, a * tc * BL + hf * PA_N:
                                    a * tc * BL + (hf + 1) * PA_N],
                                start=(ki == 0), stop=(ki == KT - 1))
                        bcol = bias_sb[:, d * G + j:d * G + j + 1]
                        dst = xp[d][:, j * tc * BL + hf * PA_N:
                                    j * tc * BL + (hf + 1) * PA_N]
                        if j % 2 == 0:
                            nc.scalar.activation(dst, ps[:], AF.Identity,
                                                 bias=bcol)
                        else:
                            nc.vector.tensor_scalar(dst, ps[:], bcol, None,
                                                    OP.add)
            # 3) sequential scan, two chains (directions) interleaved
            for tl in range(tc):
                tj = 0 if sever_h else tl
                psg = [None, None]
                xpv = [xp[d][:].rearrange("p (g t l) -> p g t l",
                                          g=G, l=BL)[:, :, tl, :]
                       for d in range(ND)]
                for d in range(ND):
                    if skip_mm:
                        continue
                    ps = pgpool.tile([128, G * BL], f32, tag=f"g{d}",
                                     name="psg")
                    psg[d] = ps
                    # inject xp into the gate bank ahead of the h MMs
                    nc.tensor.matmul(
                        ps[:].rearrange("p (g l) -> p g l", l=BL),
                        ident_sb[:], xpv[d],
                        start=True, stop=False, skip_group_check=True)
                    for j in range(G):
                        for ki in range(KT):
                            nc.tensor.matmul(
                                ps[:, j * BL:(j + 1) * BL],
                                w_sl(whh_sb, d, ki, j),
                                h_sl(d, tj, ki, BL),
                                start=False,
                                stop=(ki == KT - 1 and j == G - 1),
                                skip_group_check=True)
                if skip_eltwise:
                    continue
                sig, sigc, cf, u2 = [], [], [], []
                for d in range(ND):
                    sig.append(tpool.tile([128, G * BL], f32, tag=f"sg{d}",
                                          name="sig"))
                    sigc.append(tpool.tile([128, B2], f32, tag=f"sc{d}",
                                           name="sigc"))
                    cf.append(tpool.tile([128, B2], f32, tag=f"cf{d}",
                                         name="cf"))
                    u2.append(tpool.tile([128, B2], f32, tag=f"u{d}",
                                         name="u2"))
                src = [xpv[d] if skip_mm else psg[d][:] for d in range(ND)]

                def emit_sg(d):
                    if o_last and not skip_mm:
                        nc.scalar.activation(sig[d][:, 0:3 * B2],
                                             src[d][:, 0:3 * B2], AF.Sigmoid)
                        nc.scalar.activation(sig[d][:, 3 * B2:4 * B2],
                                             src[d][:, 3 * B2:4 * B2],
                                             AF.Sigmoid)
                    else:
                        nc.scalar.activation(sig[d][:], src[d], AF.Sigmoid)

                def emit_cchain(d):
                    def _cf():
                        nc.vector.tensor_mul(cf[d][:], sig[d][:, 0:B2],
                                             c_sl(d, tl))

                    def _u2():
                        nc.vector.scalar_tensor_tensor(
                            u2[d][:], sig[d][:, B2:2 * B2], 0.5,
                            sig[d][:, 2 * B2:3 * B2], OP.subtract, OP.mult)

                    if u2_first:
                        _u2(); _cf()
                    else:
                        _cf(); _u2()
                    nc.vector.scalar_tensor_tensor(
                        c_sl(d, tl + 1), u2[d][:], 4.0, cf[d][:],
                        OP.mult, OP.add)

                def emit_sc(d):
                    nc.scalar.activation(sigc[d][:], c_sl(d, tl + 1),
                                         AF.Sigmoid)

                def emit_h2(d):
                    if split_h2:
                        for ki in range(KT):
                            nc.vector.scalar_tensor_tensor(
                                h_sl(d, tl + 1, ki, BL),
                                sigc[d][:, ki * BL:(ki + 1) * BL], 0.5,
                                sig[d][:, 3 * B2 + ki * BL:
                                     3 * B2 + (ki + 1) * BL],
                                OP.subtract, OP.mult)
                    else:
                        nc.vector.scalar_tensor_tensor(
                            h_sl(d, tl + 1), sigc[d][:], 0.5,
                            sig[d][:, 3 * B2:4 * B2], OP.subtract, OP.mult)

                if dve_order == 0:
                    # DVE: cf0,u20,C'0, cf1,u21,C'1, h20, h21
                    emit_sg(0); emit_cchain(0); emit_sg(1); emit_sc(0)
                    emit_cchain(1); emit_h2(0); emit_sc(1); emit_h2(1)
                else:
                    # DVE: cf0,u20,C'0, h20, cf1,u21,C'1, h21
                    emit_sg(0); emit_cchain(0); emit_sg(1); emit_sc(0)
                    emit_h2(0); emit_cchain(1); emit_sc(1); emit_h2(1)
            # 4) flush chunk outputs (one DMA each for h and c), carry state
            nc.sync.dma_start(
                out=h_out.ap()[ds(kbase, RPC), :]
                    .rearrange("(a p) n -> p a n", p=128),
                in_=h_hist[:].rearrange("p (a n) -> p a n", a=ND)[:, :, B2:])
            nc.sync.dma_start(
                out=c_out.ap()[ds(kbase, RPC), :]
                    .rearrange("(a p) n -> p a n", p=128),
                in_=c_hist[:].rearrange("p (a n) -> p a n", a=ND)[:, :, B2:])
            for d in range(ND):
                nc.vector.tensor_copy(h_sl(d, 0), h_sl(d, tc))
                nc.vector.tensor_copy(c_sl(d, 0), c_sl(d, tc))

        if n_chunks == 1:
            chunk_body(0)
        else:
            import concourse.mybir as _mb
            with tcx.For_i(0, n_chunks * RPC, RPC,
                           hint_engines=(_mb.EngineType.PE,
                                         _mb.EngineType.Activation,
                                         _mb.EngineType.DVE)) as kbase:
                chunk_body(kbase)

    nc.compile()
    return nc


# ---------------- host-side data marshalling ----------------

def _perm_scale_rows(w):
    """Reorder gate rows [i,f,g,o] -> [f,g,i,o], scale g rows by 2."""
    return np.concatenate(
        [w[256:512], 2.0 * w[512:768], w[0:256], w[768:1024]], 0)


def prep_inputs(x, length, h0, c0, Wih_f, Whh_f, bih_f, bhh_f,
                Wih_b, Whh_b, bih_b, bhh_b, t_total=T, tc=TC):
    """Build per-core input dicts."""
    n_chunks = t_total // tc
    dt = np.float16
    x = np.asarray(x, np.float32)
    x_b = x[::-1, ::-1, :]

    # Whh additionally scaled by 2: the h state is stored as h/2.
    wihP = {0: _perm_scale_rows(np.asarray(Wih_f)),
            1: _perm_scale_rows(np.asarray(Wih_b))}
    whhP = {0: 2.0 * _perm_scale_rows(np.asarray(Whh_f)),
            1: 2.0 * _perm_scale_rows(np.asarray(Whh_b))}
    biasP = {0: _perm_scale_rows(
                 (np.asarray(bih_f) + np.asarray(bhh_f))[:, None]),
             1: _perm_scale_rows(
                 (np.asarray(bih_b) + np.asarray(bhh_b))[:, None])}

    def wtiles(w):
        out = np.empty((ND * KT * G * 128, 128), dt)
        for d in range(ND):
            wT = w[d].T.astype(dt)
            for ki in range(KT):
                for j in range(G):
                    off = ((d * KT + ki) * G + j) * 128
                    out[off:off + 128] = wT[ki * 128:(ki + 1) * 128,
                                            j * 128:(j + 1) * 128]
        return out

    whhT = wtiles(whhP)
    wihT = wtiles(wihP)
    biasT = np.zeros((128, ND * G), np.float32)
    for d in range(ND):
        for j in range(G):
            biasT[:, d * G + j] = biasP[d][j * 128:(j + 1) * 128, 0]

    h0 = np.asarray(h0, np.float32)
    c0 = np.asarray(c0, np.float32)

    in_maps = []
    for core in range(NCORES):
        sl = slice(core * BL, (core + 1) * BL)
        xarr = {}
        for d, xd in ((0, x), (1, x_b)):
            xs = xd[:t_total, sl, :]
            xT = np.ascontiguousarray(xs.transpose(0, 2, 1)).astype(dt)
            xa = np.empty((n_chunks * RPC, tc * BL), dt)
            for k in range(n_chunks):
                for ki in range(KT):
                    roff = k * RPC + ki * 128
                    blk = xT[k * tc:(k + 1) * tc,
                             ki * 128:(ki + 1) * 128, :]
                    xa[roff:roff + 128] = (
                        blk.transpose(1, 0, 2).reshape(128, tc * BL))
            xarr[d] = xa
        hc0T = np.zeros((128, ND * 2 * B2), np.float32)
        for d in range(ND):
            for s, st in ((0, 0.5 * h0), (1, 2.0 * c0)):
                stT = st[sl].T
                for ki in range(KT):
                    off = (2 * d + s) * B2 + ki * BL
                    hc0T[:, off:off + BL] = stT[ki * 128:(ki + 1) * 128, :]
        in_maps.append({"xarr0": xarr[0], "xarr1": xarr[1], "whhT": whhT,
                        "wihT": wihT, "biasT": biasT, "hc0T": hc0T,
                        "identT": np.eye(128, dtype=dt)})
    return in_maps


def assemble_outputs(results, length, t_total=T, tc=TC):
    """results: per-core {'h_out','c_out'}. Returns (output, cell)."""
    n_chunks = t_total // tc
    length = np.asarray(length)
    out_h = np.empty((t_total, 2 * B, H), np.float32)
    out_c = np.empty((t_total, 2 * B, H), np.float32)
    for core in range(NCORES):
        sl = slice(core * BL, (core + 1) * BL)
        for arr, out, scale in ((results[core]["h_out"], out_h, 2.0),
                                (results[core]["c_out"], out_c, 0.5)):
            v = arr.astype(np.float32).reshape(n_chunks, ND, 128, tc, KT, BL)
            # [k, d, p, tl, ki, l] -> [d, (k tl), l, (ki p)]
            v = v.transpose(1, 0, 3, 5, 4, 2).reshape(ND, t_total, BL, H)
            for d in range(ND):
                col0 = d * B + sl.start
                out[:, col0:col0 + BL, :] = scale * v[d]
    for b in range(B):
        ln = int(length[b])
        if ln < t_total:
            out_h[ln:, b] = out_h[ln - 1, b]
            out_c[ln:, b] = out_c[ln - 1, b]
            out_h[ln:, B + b] = out_h[ln - 1, B + b]
            out_c[ln:, B + b] = out_c[ln - 1, B + b]
    return out_h, out_c


def _get_exec(nc, n_cores=NCORES):
    """jit-once PJRT runner (the run_bass_via_pjrt multi-core path, with
    the jitted executable cached across kernel() calls)."""
    import jax
    import jax.numpy as jnp
    from jax.sharding import Mesh, PartitionSpec, NamedSharding
    from concourse import bass2jax, mybir
    try:
        from jax.experimental.shard_map import shard_map
    except ImportError:
        from jax.shard_map import shard_map

    bass2jax.install_neuronx_cc_hook()

    partition_name = (nc.partition_id_tensor.name
                      if nc.partition_id_tensor else None)
    in_names, out_names, out_avals, zero_shapes = [], [], [], []
    for alloc in nc.m.functions[0].allocations:
        if not isinstance(alloc, mybir.MemoryLocationSet):
            continue
        name = alloc.memorylocations[0].name
        if alloc.kind == "ExternalInput":
            if name != partition_name:
                in_names.append(name)
        elif alloc.kind == "ExternalOutput":
            out_names.append(name)
            shape = tuple(alloc.tensor_shape)
            dtype = mybir.dt.np(alloc.dtype)
            out_avals.append(jax.core.ShapedArray(shape, dtype))
            zero_shapes.append((shape, dtype))
    n_params = len(in_names)
    n_outs = len(out_names)
    all_in_names = in_names + out_names
    if partition_name is not None:
        all_in_names = all_in_names + [partition_name]
    donate = tuple(range(n_params, n_params + n_outs))

    def _body(*args):
        operands = list(args)
        if partition_name is not None:
            operands.append(bass2jax.partition_id_tensor())
        outs = bass2jax._bass_exec_p.bind(
            *operands,
            out_avals=tuple(out_avals),
            in_names=tuple(all_in_names),
            out_names=tuple(out_names),
            lowering_input_output_aliases=(),
            sim_require_finite=True,
            sim_require_nnan=True,
            nc=nc,
        )
        return tuple(outs)

    devices = jax.devices()[:n_cores]
    mesh = Mesh(np.asarray(devices), ("core",))
    spec = PartitionSpec("core")
    fn = jax.jit(
        shard_map(_body, mesh=mesh, in_specs=(spec,) * (n_params + n_outs),
                  out_specs=(spec,) * n_outs, check_rep=False),
        donate_argnums=donate, keep_unused=True)
    sharding = NamedSharding(mesh, spec)
    zmaker = jax.jit(
        lambda: tuple(
            jnp.zeros((n_cores * s[0], *s[1:]), d) for s, d in zero_shapes),
        out_shardings=(sharding,) * n_outs)

    def prep_dev(in_maps):
        import jax as _jax
        concat_in = [
            np.concatenate([np.asarray(in_maps[c][nm])
                            for c in range(n_cores)], axis=0)
            for nm in in_names
        ]
        return [_jax.device_put(a, sharding) for a in concat_in]

    def exec_dev(ins_dev, donated=None):
        zs = donated if donated is not None else zmaker()
        return fn(*ins_dev, *zs)

    def run(in_maps, donated=None):
        return exec_dev(prep_dev(in_maps), donated)

    def get_outputs(res):
        return [
            {nm: np.asarray(res[i]).reshape(n_cores, *out_avals[i].shape)[c]
             for i, nm in enumerate(out_names)}
            for c in range(n_cores)
        ]

    return run, get_outputs, prep_dev, exec_dev


def kernel(**inputs):
    _import_bass()
    key = (T, TC)
    if key not in _CACHE:
        nc = build_program(T, TC)
        _CACHE[key] = (nc,) + _get_exec(nc)
    nc, run, get_outputs = _CACHE[key][:3]
    in_maps = prep_inputs(**inputs)
    res = run(in_maps)
    results = get_outputs(res)
    return assemble_outputs(results, inputs["length"])


# revision 15
# speedup vs baseline: 2.2532x; 1.0267x over previous
"""BiLSTM (T=2048, B=32, I=H=256) Bass kernel for 8 NeuronCores.

Sharding (data-parallel per the hint): batch is split 8 ways; every core
runs BOTH directions for its 4 batch lanes as two independent chains so
each chain's cross-engine latency is hidden by the other chain's work.
The backward direction consumes x flipped along time AND batch (faithful
to torch.flip(input_, [0,1])).

On-chip layout is fully transposed ([H partitions, lanes free]) so the
sequential scan needs no per-step transposes. The recurrent step is
PE-issue-bound on HW (~44ns/matmul instr x 34 instrs/step), so the
elementwise path is cut to the minimum: a sigmoid-only formulation

    store  h~ = h/2   (Whh pre-scaled by 2 on host)
    store  C  = 2c    (c0 pre-scaled, output halved on host)
    gate rows permuted to [f,g,i,o], g rows pre-scaled by 2

    sig    = Sigmoid(gates)                      1 Act instr  [128,32]
    cf     = sig_f * C                           1 DVE tt
    u2     = (sig_g - 0.5) * sig_i  (= u/2)      1 DVE stt
    C'     = 4*u2 + cf              (= 2c')      1 DVE stt
    sigc   = Sigmoid(C')                         1 Act instr  [128,8]
    h~'    = (sigc - 0.5) * sig_o   (= h'/2)     1 DVE stt

which is algebraically exact (tanh(x) = 2*sigmoid(2x) - 1) and needs no
Tanh activation table. Weights/x/h run in fp16; c stays fp32 on chip.

Length masking is exact and handled on the host: a lane's post-length
steps compute garbage that never contaminates other lanes (lanes are
independent columns end to end; sigmoids keep values bounded), and the
output tail t >= len is overwritten host-side with the frozen value at
len-1 — identical to the reference's masked freeze.
"""

import sys

import numpy as np

# ---- problem constants (hardcoded per contract) ----
T, B, I, H = 2048, 32, 256, 256
NCORES = 8
ND = 2            # directions per core
BL = B // NCORES  # 4 batch lanes per core per direction
B2 = 2 * BL       # (H-tile, lane) free width of h/c state
G = 8             # 4H/128 gate row tiles, order [f0,f1,g0,g1,i0,i1,o0,o1]
KT = 2            # H/128 contraction tiles
TC = 256          # scan chunk length (steps per For_i iteration)
RPC = KT * 128    # chunk row stride shared by xarr0/1, h_out, c_out

_CACHE = {}


def _import_bass():
    try:
        import concourse.bass  # noqa: F401
    except ImportError:
        sys.path.insert(0, "/opt/trn_rl_repo")


def build_program(t_total=T, tc=TC,
                  skip_mm=False, skip_eltwise=False, sever_h=False,
                  dve_order=1, split_h2=False, u2_first=True, o_last=False):
    """Build the SPMD Bass program (identical on all cores)."""
    _import_bass()
    import concourse.bass as bass
    import concourse.mybir as mybir
    from concourse import bacc
    from concourse.tile import TileContext

    ds = bass.ds
    f32 = mybir.dt.float32
    f16 = mybir.dt.float16
    AF = mybir.ActivationFunctionType
    OP = mybir.AluOpType

    n_chunks = t_total // tc
    assert t_total % tc == 0

    nc = bacc.Bacc("TRN2", target_bir_lowering=False, debug=False,
                   num_devices=NCORES)

    # DRAM I/O. All chunked tensors share a 256-row-per-chunk stride so
    # one For_i induction variable addresses everything.
    xarr0 = nc.dram_tensor("xarr0", [n_chunks * RPC, tc * BL], f16,
                           kind="ExternalInput")
    xarr1 = nc.dram_tensor("xarr1", [n_chunks * RPC, tc * BL], f16,
                           kind="ExternalInput")
    whhT = nc.dram_tensor("whhT", [ND * KT * G * 128, 128], f16,
                          kind="ExternalInput")
    wihT = nc.dram_tensor("wihT", [ND * KT * G * 128, 128], f16,
                          kind="ExternalInput")
    biasT = nc.dram_tensor("biasT", [128, ND * G], f32, kind="ExternalInput")
    hc0T = nc.dram_tensor("hc0T", [128, ND * 2 * B2], f32,
                          kind="ExternalInput")
    identT = nc.dram_tensor("identT", [128, 128], f16, kind="ExternalInput")
    h_out = nc.dram_tensor("h_out", [n_chunks * RPC, tc * B2], f16,
                           kind="ExternalOutput")
    c_out = nc.dram_tensor("c_out", [n_chunks * RPC, tc * B2], f32,
                           kind="ExternalOutput")

    from contextlib import ExitStack
    with TileContext(nc) as tcx, ExitStack() as stk:
        wpool = stk.enter_context(tcx.tile_pool(name="weights", bufs=1))
        spool = stk.enter_context(tcx.tile_pool(name="state", bufs=1))
        xpool = stk.enter_context(tcx.tile_pool(name="xdata", bufs=1))
        tpool = stk.enter_context(tcx.tile_pool(name="temps", bufs=3))
        papool = stk.enter_context(tcx.tile_pool(name="psa", bufs=2,
                                                 space="PSUM"))
        pgpool = stk.enter_context(tcx.tile_pool(name="psg", bufs=2,
                                                 space="PSUM"))

        whh_sb = wpool.tile([128, ND * KT * G * 128], f16)
        wih_sb = wpool.tile([128, ND * KT * G * 128], f16)
        bias_sb = wpool.tile([128, ND * G], f32)
        hc0_sb = wpool.tile([128, ND * 2 * B2], f32)
        ident_sb = wpool.tile([128, 128], f16)
        HB = (tc + 1) * B2  # per-direction history block
        h_hist = spool.tile([128, ND * HB], f16)
        c_hist = spool.tile([128, ND * HB], f32)
        xp = [xpool.tile([128, G * tc * BL], f16, name=f"xp{d}")
              for d in range(ND)]
        xin = xpool.tile([128, ND * KT * tc * BL], f16)

        def w_sl(sb, d, ki, j):
            off = ((d * KT + ki) * G + j) * 128
            return sb[:, off:off + 128]

        def h_sl(d, slot, ki=0, w=None):
            off = d * HB + slot * B2 + ki * BL
            return h_hist[:, off:off + (w if w is not None else B2)]

        def c_sl(d, slot):
            off = d * HB + slot * B2
            return c_hist[:, off:off + B2]

        # --- load constants ---
        nc.sync.dma_start(
            out=whh_sb[:].rearrange("p (a m) -> p a m", m=128),
            in_=whhT.ap().rearrange("(a p) m -> p a m", p=128))
        nc.sync.dma_start(
            out=wih_sb[:].rearrange("p (a m) -> p a m", m=128),
            in_=wihT.ap().rearrange("(a p) m -> p a m", p=128))
        nc.sync.dma_start(out=bias_sb[:], in_=biasT.ap())
        nc.sync.dma_start(out=hc0_sb[:], in_=hc0T.ap())
        nc.sync.dma_start(out=ident_sb[:], in_=identT.ap())
        for d in range(ND):
            nc.vector.tensor_copy(h_sl(d, 0),
                                  hc0_sb[:, (2 * d) * B2:(2 * d + 1) * B2])
            nc.vector.tensor_copy(c_sl(d, 0),
                                  hc0_sb[:, (2 * d + 1) * B2:(2 * d + 2) * B2])

        def chunk_body(kbase):
            # 1) DMA x.T chunk in, one transfer per direction
            for d, xa in ((0, xarr0), (1, xarr1)):
                nc.sync.dma_start(
                    out=xin[:, d * KT * tc * BL:(d + 1) * KT * tc * BL]
                        .rearrange("p (a n) -> p a n", a=KT),
                    in_=xa.ap()[ds(kbase, RPC), :]
                        .rearrange("(a p) n -> p a n", p=128))
            # 2) Phase A: xp[d] = Wih_perm @ x.T + bias, laid out (j, t, l)
            PA_N = min(tc * BL, 512)
            for d in range(ND):
                for j in range(G):
                    for hf in range(tc * BL // PA_N):
                        ps = papool.tile([128, PA_N], f32, tag=f"pa{d}",
                                         name="psa")
                        for ki in range(KT):
                            a = d * KT + ki
                            nc.tensor.matmul(
                                ps[:], w_sl(wih_sb, d, ki, j),
                                xin[:, a * tc * BL + hf * PA_N:
                                    a * tc * BL + (hf + 1) * PA_N],
                                start=(ki == 0), stop=(ki == KT - 1))
                        bcol = bias_sb[:, d * G + j:d * G + j + 1]
                        dst = xp[d][:, j * tc * BL + hf * PA_N:
                                    j * tc * BL + (hf + 1) * PA_N]
                        # all bias adds on DVE: keeps the Act queue free for
                        # the first scan steps' sigmoids at each chunk start
                        nc.vector.tensor_scalar(dst, ps[:], bcol, None,
                                                OP.add)
            # 3) sequential scan, two chains (directions) interleaved
            for tl in range(tc):
                tj = 0 if sever_h else tl
                psg = [None, None]
                xpv = [xp[d][:].rearrange("p (g t l) -> p g t l",
                                          g=G, l=BL)[:, :, tl, :]
                       for d in range(ND)]
                for d in range(ND):
                    if skip_mm:
                        continue
                    ps = pgpool.tile([128, G * BL], f32, tag=f"g{d}",
                                     name="psg")
                    psg[d] = ps
                    # inject xp into the gate bank ahead of the h MMs
                    nc.tensor.matmul(
                        ps[:].rearrange("p (g l) -> p g l", l=BL),
                        ident_sb[:], xpv[d],
                        start=True, stop=False, skip_group_check=True)
                    for j in range(G):
                        for ki in range(KT):
                            nc.tensor.matmul(
                                ps[:, j * BL:(j + 1) * BL],
                                w_sl(whh_sb, d, ki, j),
                                h_sl(d, tj, ki, BL),
                                start=False,
                                stop=(ki == KT - 1 and j == G - 1),
                                skip_group_check=True)
                if skip_eltwise:
                    continue
                sig, sigc, cf, u2 = [], [], [], []
                for d in range(ND):
                    sig.append(tpool.tile([128, G * BL], f32, tag=f"sg{d}",
                                          name="sig"))
                    sigc.append(tpool.tile([128, B2], f32, tag=f"sc{d}",
                                           name="sigc"))
                    cf.append(tpool.tile([128, B2], f32, tag=f"cf{d}",
                                         name="cf"))
                    u2.append(tpool.tile([128, B2], f32, tag=f"u{d}",
                                         name="u2"))
                src = [xpv[d] if skip_mm else psg[d][:] for d in range(ND)]

                def emit_sg(d):
                    if o_last and not skip_mm:
                        nc.scalar.activation(sig[d][:, 0:3 * B2],
                                             src[d][:, 0:3 * B2], AF.Sigmoid)
                        nc.scalar.activation(sig[d][:, 3 * B2:4 * B2],
                                             src[d][:, 3 * B2:4 * B2],
                                             AF.Sigmoid)
                    else:
                        nc.scalar.activation(sig[d][:], src[d], AF.Sigmoid)

                def emit_cchain(d):
                    def _cf():
                        nc.vector.tensor_mul(cf[d][:], sig[d][:, 0:B2],
                                             c_sl(d, tl))

                    def _u2():
                        nc.vector.scalar_tensor_tensor(
                            u2[d][:], sig[d][:, B2:2 * B2], 0.5,
                            sig[d][:, 2 * B2:3 * B2], OP.subtract, OP.mult)

                    if u2_first:
                        _u2(); _cf()
                    else:
                        _cf(); _u2()
                    nc.vector.scalar_tensor_tensor(
                        c_sl(d, tl + 1), u2[d][:], 4.0, cf[d][:],
                        OP.mult, OP.add)

                def emit_sc(d):
                    nc.scalar.activation(sigc[d][:], c_sl(d, tl + 1),
                                         AF.Sigmoid)

                def emit_h2(d):
                    if split_h2:
                        for ki in range(KT):
                            nc.vector.scalar_tensor_tensor(
                                h_sl(d, tl + 1, ki, BL),
                                sigc[d][:, ki * BL:(ki + 1) * BL], 0.5,
                                sig[d][:, 3 * B2 + ki * BL:
                                     3 * B2 + (ki + 1) * BL],
                                OP.subtract, OP.mult)
                    else:
                        nc.vector.scalar_tensor_tensor(
                            h_sl(d, tl + 1), sigc[d][:], 0.5,
                            sig[d][:, 3 * B2:4 * B2], OP.subtract, OP.mult)

                if dve_order == 0:
                    # DVE: cf0,u20,C'0, cf1,u21,C'1, h20, h21
                    emit_sg(0); emit_cchain(0); emit_sg(1); emit_sc(0)
                    emit_cchain(1); emit_h2(0); emit_sc(1); emit_h2(1)
                else:
                    # DVE: cf0,u20,C'0, h20, cf1,u21,C'1, h21
                    emit_sg(0); emit_cchain(0); emit_sg(1); emit_sc(0)
                    emit_h2(0); emit_cchain(1); emit_sc(1); emit_h2(1)
            # 4) flush chunk outputs (one DMA each for h and c), carry state
            nc.sync.dma_start(
                out=h_out.ap()[ds(kbase, RPC), :]
                    .rearrange("(a p) n -> p a n", p=128),
                in_=h_hist[:].rearrange("p (a n) -> p a n", a=ND)[:, :, B2:])
            nc.sync.dma_start(
                out=c_out.ap()[ds(kbase, RPC), :]
                    .rearrange("(a p) n -> p a n", p=128),
                in_=c_hist[:].rearrange("p (a n) -> p a n", a=ND)[:, :, B2:])
            for d in range(ND):
                nc.vector.tensor_copy(h_sl(d, 0), h_sl(d, tc))
                nc.vector.tensor_copy(c_sl(d, 0), c_sl(d, tc))

        if n_chunks == 1:
            chunk_body(0)
        else:
            import concourse.mybir as _mb
            with tcx.For_i(0, n_chunks * RPC, RPC,
                           hint_engines=(_mb.EngineType.PE,
                                         _mb.EngineType.Activation,
                                         _mb.EngineType.DVE)) as kbase:
                chunk_body(kbase)

    nc.compile()
    return nc


# ---------------- host-side data marshalling ----------------

def _perm_scale_rows(w):
    """Reorder gate rows [i,f,g,o] -> [f,g,i,o], scale g rows by 2."""
    return np.concatenate(
        [w[256:512], 2.0 * w[512:768], w[0:256], w[768:1024]], 0)


def prep_inputs(x, length, h0, c0, Wih_f, Whh_f, bih_f, bhh_f,
                Wih_b, Whh_b, bih_b, bhh_b, t_total=T, tc=TC):
    """Build per-core input dicts."""
    n_chunks = t_total // tc
    dt = np.float16
    x = np.asarray(x, np.float32)
    x_b = x[::-1, ::-1, :]

    # Whh additionally scaled by 2: the h state is stored as h/2.
    wihP = {0: _perm_scale_rows(np.asarray(Wih_f)),
            1: _perm_scale_rows(np.asarray(Wih_b))}
    whhP = {0: 2.0 * _perm_scale_rows(np.asarray(Whh_f)),
            1: 2.0 * _perm_scale_rows(np.asarray(Whh_b))}
    biasP = {0: _perm_scale_rows(
                 (np.asarray(bih_f) + np.asarray(bhh_f))[:, None]),
             1: _perm_scale_rows(
                 (np.asarray(bih_b) + np.asarray(bhh_b))[:, None])}

    def wtiles(w):
        out = np.empty((ND * KT * G * 128, 128), dt)
        for d in range(ND):
            wT = w[d].T.astype(dt)
            for ki in range(KT):
                for j in range(G):
                    off = ((d * KT + ki) * G + j) * 128
                    out[off:off + 128] = wT[ki * 128:(ki + 1) * 128,
                                            j * 128:(j + 1) * 128]
        return out

    whhT = wtiles(whhP)
    wihT = wtiles(wihP)
    biasT = np.zeros((128, ND * G), np.float32)
    for d in range(ND):
        for j in range(G):
            biasT[:, d * G + j] = biasP[d][j * 128:(j + 1) * 128, 0]

    h0 = np.asarray(h0, np.float32)
    c0 = np.asarray(c0, np.float32)

    in_maps = []
    for core in range(NCORES):
        sl = slice(core * BL, (core + 1) * BL)
        xarr = {}
        for d, xd in ((0, x), (1, x_b)):
            xs = xd[:t_total, sl, :]
            xT = np.ascontiguousarray(xs.transpose(0, 2, 1)).astype(dt)
            xa = np.empty((n_chunks * RPC, tc * BL), dt)
            for k in range(n_chunks):
                for ki in range(KT):
                    roff = k * RPC + ki * 128
                    blk = xT[k * tc:(k + 1) * tc,
                             ki * 128:(ki + 1) * 128, :]
                    xa[roff:roff + 128] = (
                        blk.transpose(1, 0, 2).reshape(128, tc * BL))
            xarr[d] = xa
        hc0T = np.zeros((128, ND * 2 * B2), np.float32)
        for d in range(ND):
            for s, st in ((0, 0.5 * h0), (1, 2.0 * c0)):
                stT = st[sl].T
                for ki in range(KT):
                    off = (2 * d + s) * B2 + ki * BL
                    hc0T[:, off:off + BL] = stT[ki * 128:(ki + 1) * 128, :]
        in_maps.append({"xarr0": xarr[0], "xarr1": xarr[1], "whhT": whhT,
                        "wihT": wihT, "biasT": biasT, "hc0T": hc0T,
                        "identT": np.eye(128, dtype=dt)})
    return in_maps


def assemble_outputs(results, length, t_total=T, tc=TC):
    """results: per-core {'h_out','c_out'}. Returns (output, cell)."""
    n_chunks = t_total // tc
    length = np.asarray(length)
    out_h = np.empty((t_total, 2 * B, H), np.float32)
    out_c = np.empty((t_total, 2 * B, H), np.float32)
    for core in range(NCORES):
        sl = slice(core * BL, (core + 1) * BL)
        for arr, out, scale in ((results[core]["h_out"], out_h, 2.0),
                                (results[core]["c_out"], out_c, 0.5)):
            v = arr.astype(np.float32).reshape(n_chunks, ND, 128, tc, KT, BL)
            # [k, d, p, tl, ki, l] -> [d, (k tl), l, (ki p)]
            v = v.transpose(1, 0, 3, 5, 4, 2).reshape(ND, t_total, BL, H)
            for d in range(ND):
                col0 = d * B + sl.start
                out[:, col0:col0 + BL, :] = scale * v[d]
    for b in range(B):
        ln = int(length[b])
        if ln < t_total:
            out_h[ln:, b] = out_h[ln - 1, b]
            out_c[ln:, b] = out_c[ln - 1, b]
            out_h[ln:, B + b] = out_h[ln - 1, B + b]
            out_c[ln:, B + b] = out_c[ln - 1, B + b]
    return out_h, out_c


def _get_exec(nc, n_cores=NCORES):
    """jit-once PJRT runner (the run_bass_via_pjrt multi-core path, with
    the jitted executable cached across kernel() calls)."""
    import jax
    import jax.numpy as jnp
    from jax.sharding import Mesh, PartitionSpec, NamedSharding
    from concourse import bass2jax, mybir
    try:
        from jax.experimental.shard_map import shard_map
    except ImportError:
        from jax.shard_map import shard_map

    bass2jax.install_neuronx_cc_hook()

    partition_name = (nc.partition_id_tensor.name
                      if nc.partition_id_tensor else None)
    in_names, out_names, out_avals, zero_shapes = [], [], [], []
    for alloc in nc.m.functions[0].allocations:
        if not isinstance(alloc, mybir.MemoryLocationSet):
            continue
        name = alloc.memorylocations[0].name
        if alloc.kind == "ExternalInput":
            if name != partition_name:
                in_names.append(name)
        elif alloc.kind == "ExternalOutput":
            out_names.append(name)
            shape = tuple(alloc.tensor_shape)
            dtype = mybir.dt.np(alloc.dtype)
            out_avals.append(jax.core.ShapedArray(shape, dtype))
            zero_shapes.append((shape, dtype))
    n_params = len(in_names)
    n_outs = len(out_names)
    all_in_names = in_names + out_names
    if partition_name is not None:
        all_in_names = all_in_names + [partition_name]
    donate = tuple(range(n_params, n_params + n_outs))

    def _body(*args):
        operands = list(args)
        if partition_name is not None:
            operands.append(bass2jax.partition_id_tensor())
        outs = bass2jax._bass_exec_p.bind(
            *operands,
            out_avals=tuple(out_avals),
            in_names=tuple(all_in_names),
            out_names=tuple(out_names),
            lowering_input_output_aliases=(),
            sim_require_finite=True,
            sim_require_nnan=True,
            nc=nc,
        )
        return tuple(outs)

    devices = jax.devices()[:n_cores]
    mesh = Mesh(np.asarray(devices), ("core",))
    spec = PartitionSpec("core")
    fn = jax.jit(
        shard_map(_body, mesh=mesh, in_specs=(spec,) * (n_params + n_outs),
                  out_specs=(spec,) * n_outs, check_rep=False),
        donate_argnums=donate, keep_unused=True)
    sharding = NamedSharding(mesh, spec)
    zmaker = jax.jit(
        lambda: tuple(
            jnp.zeros((n_cores * s[0], *s[1:]), d) for s, d in zero_shapes),
        out_shardings=(sharding,) * n_outs)

    def prep_dev(in_maps):
        import jax as _jax
        concat_in = [
            np.concatenate([np.asarray(in_maps[c][nm])
                            for c in range(n_cores)], axis=0)
            for nm in in_names
        ]
        return [_jax.device_put(a, sharding) for a in concat_in]

    def exec_dev(ins_dev, donated=None):
        zs = donated if donated is not None else zmaker()
        return fn(*ins_dev, *zs)

    def run(in_maps, donated=None):
        return exec_dev(prep_dev(in_maps), donated)

    def get_outputs(res):
        return [
            {nm: np.asarray(res[i]).reshape(n_cores, *out_avals[i].shape)[c]
             for i, nm in enumerate(out_names)}
            for c in range(n_cores)
        ]

    return run, get_outputs, prep_dev, exec_dev


def kernel(**inputs):
    _import_bass()
    key = (T, TC)
    if key not in _CACHE:
        nc = build_program(T, TC)
        _CACHE[key] = (nc,) + _get_exec(nc)
    nc, run, get_outputs = _CACHE[key][:3]
    in_maps = prep_inputs(**inputs)
    res = run(in_maps)
    results = get_outputs(res)
    return assemble_outputs(results, inputs["length"])
